# revision 7
# baseline (speedup 1.0000x reference)
"""Trainium2 Bass kernel for nn_Attention_27874337751091.

Dense single-head attention block (GroupNorm -> qkv 1x1 conv -> softmax
attention over N=H*W tokens -> proj with residual-to-attention-output).

Sharding: data-parallel over batch B=16 across 8 NeuronCores (2 batches per
core). Weights replicated; no collectives. Each core runs an identical
program on its batch shard; the host gathers by concatenation.

Per-core pipeline (per batch, activations kept as [C, N] channel-major):
  - GroupNorm stats via bn_stats/bn_aggr per channel + a tiny PE matmul with
    a block-diagonal group-averaging matrix to broadcast group stats back to
    channels, then one fused scale+shift DVE pass.
  - q,k in [c,n] layout, v directly transposed to [m,c] (by swapping matmul
    operands), so the attention output matmul needs no extra transpose of v.
  - S = (q*C^-1/4-ish scale folded into q) ^T k accumulated in PSUM, exp on
    ScalarE with accumulated row-sum, rows pre-scaled by 1/sum, P stored
    bf16 and transposed 128x128 via DMA-xbar for the attn@v matmul.
  - proj residual is folded into the proj weight on the host (W' = W + I).
"""

from contextlib import ExitStack

import numpy as np

import concourse.bass as bass
import concourse.mybir as mybir
import concourse.tile as tile
from concourse.vector_clock import ScopedClock

# ---------------------------------------------------------------------------
# Problem constants (hardcoded per the grading contract)
# ---------------------------------------------------------------------------
N_CORES = 8
B, C, H, W = 16, 512, 32, 32
N = H * W                      # 1024 tokens
BL = B // N_CORES              # 2 batches per core
G = 32                         # groupnorm groups
GS = C // G                    # 16 channels per group
EPS = 1e-5
P = 128                        # partitions
CT = C // P                    # 4 channel tiles
NT = N // P                    # 8 token tiles
NCH = 2                        # moving-dim chunks of 512 over N
QK_SCALE = float(C) ** -0.5

F32 = mybir.dt.float32
BF16 = mybir.dt.bfloat16

# matmul input dtype mode: "bf16" (cast activations/weights to bf16) or
# "f32r" (keep fp32 on SBUF, feed the PE with float32r-bitcast APs; the
# P/PT/vT attention-probability path stays bf16 for the DMA transpose).
MM_MODE = "bf16"


# ---------------------------------------------------------------------------
# Toolchain workaround: walrus in this environment accepts at most one
# sync-wait command on a Drain, but TileContext._drain_and_barrier attaches
# the whole global clock to a single Drain. Spread the waits over individual
# SP wait_ge instructions instead.
# ---------------------------------------------------------------------------
def _patched_drain_and_barrier(self, tick_clock, wait_clock):
    nc = self.nc
    drain_inst = nc.sync.drain()
    wait_clock.add_sem_waits(
        drain_inst.ins, ScopedClock({None: tick_clock.global_clock})
    )
    si = drain_inst.ins.sync_info
    waits = list(si.on_wait) if si is not None else []
    if len(waits) > 1:
        drain_inst.ins.sync_info = mybir.SyncInfo(
            on_wait=[waits[0]], on_update=list(si.on_update)
        )
        byname = {}
        for h in wait_clock.sems.allocated().values():
            byname[getattr(h, "name", None)] = h
        for w in waits[1:]:
            nc.sync.wait_ge(byname[w.ant_name], w.wait_value)

    nc.all_engine_barrier()
    assert self.sems is not None
    popped = nc._tile_sem_poison_stack.pop()
    assert popped is self._sem_poison
    nc.clear_and_free_semaphores(list(self.sems.allocated().values()))
    nc.all_engine_barrier()


def _apply_tile_patch():
    if not getattr(tile.TileContext, "_ant_drain_patch", False):
        tile.TileContext._drain_and_barrier = _patched_drain_and_barrier
        tile.TileContext._ant_drain_patch = True


# ---------------------------------------------------------------------------
# Kernel body
# ---------------------------------------------------------------------------
def _mm(ap):
    """View an SBUF AP in the PE input dtype for MM_MODE=f32r."""
    if MM_MODE == "f32r":
        return ap.bitcast(mybir.dt.float32r)
    return ap


def _emit(tc):
    nc = tc.nc
    mmdt = BF16 if MM_MODE == "bf16" else F32

    x_d = nc.dram_tensor("x", [BL, C, N], F32, kind="ExternalInput").ap()
    wqkv_d = nc.dram_tensor("wqkvT", [C, 3 * C], F32, kind="ExternalInput").ap()
    wproj_d = nc.dram_tensor("wprojT", [C, C], F32, kind="ExternalInput").ap()
    gnw_d = nc.dram_tensor("gnw44", [P, CT], F32, kind="ExternalInput").ap()
    gnb_d = nc.dram_tensor("gnb44", [P, CT], F32, kind="ExternalInput").ap()
    qb_d = nc.dram_tensor("qb44", [P, CT], F32, kind="ExternalInput").ap()
    kb_d = nc.dram_tensor("kb44", [P, CT], F32, kind="ExternalInput").ap()
    pb_d = nc.dram_tensor("pb44", [P, CT], F32, kind="ExternalInput").ap()
    vb_d = nc.dram_tensor("vbrow", [1, C], F32, kind="ExternalInput").ap()
    gmat_d = nc.dram_tensor("gmat", [P, P], F32, kind="ExternalInput").ap()
    y_d = nc.dram_tensor("y", [BL, C, N], F32, kind="ExternalOutput").ap()

    ctx = ExitStack()
    consts = ctx.enter_context(tc.tile_pool(name="consts", bufs=1))
    xpool = ctx.enter_context(tc.tile_pool(name="xpool", bufs=2))
    xnpool = ctx.enter_context(tc.tile_pool(name="xnpool", bufs=2))
    qkpool = ctx.enter_context(tc.tile_pool(name="qkpool", bufs=1))
    vpool = ctx.enter_context(tc.tile_pool(name="vpool", bufs=1))
    ppool = ctx.enter_context(tc.tile_pool(name="ppool", bufs=1))
    opool = ctx.enter_context(tc.tile_pool(name="opool", bufs=1))
    ypool = ctx.enter_context(tc.tile_pool(name="ypool", bufs=2))
    small = ctx.enter_context(tc.tile_pool(name="small", bufs=4))
    psum = ctx.enter_context(tc.tile_pool(name="psum", bufs=8, space="PSUM"))

    # --- constants ---
    wqkv = consts.tile([P, CT, 3 * C], mmdt, tag="wqkv")
    wqkv_src = wqkv_d.rearrange("(t p) o -> p t o", p=P)
    if mmdt == F32:
        nc.sync.dma_start(out=wqkv, in_=wqkv_src)
    else:
        nc.gpsimd.dma_start(out=wqkv, in_=wqkv_src)  # SWDGE casts f32->bf16
    wproj = consts.tile([P, CT, C], mmdt, tag="wproj")
    wproj_src = wproj_d.rearrange("(t p) o -> p t o", p=P)
    if mmdt == F32:
        nc.sync.dma_start(out=wproj, in_=wproj_src)
    else:
        nc.gpsimd.dma_start(out=wproj, in_=wproj_src)

    gnw = consts.tile([P, CT], F32, tag="gnw")
    nc.sync.dma_start(out=gnw, in_=gnw_d)
    gnb = consts.tile([P, CT], F32, tag="gnb")
    nc.sync.dma_start(out=gnb, in_=gnb_d)
    qb = consts.tile([P, CT], F32, tag="qb")
    nc.sync.dma_start(out=qb, in_=qb_d)
    kb = consts.tile([P, CT], F32, tag="kb")
    nc.sync.dma_start(out=kb, in_=kb_d)
    pb = consts.tile([P, CT], F32, tag="pb")
    nc.sync.dma_start(out=pb, in_=pb_d)
    gmat = consts.tile([P, P], F32, tag="gmat")
    nc.sync.dma_start(out=gmat, in_=gmat_d)
    epsc = consts.tile([P, 1], F32, tag="epsc")
    nc.vector.memset(epsc, EPS)
    zeroc = consts.tile([P, 1], F32, tag="zeroc")
    nc.vector.memset(zeroc, 0.0)
    # v-bias broadcast across partitions (DMA partition-step-0 replication)
    vb = consts.tile([P, C], F32, tag="vb")
    vb_bcast = bass.AP(
        tensor=vb_d.tensor,
        offset=vb_d.offset,
        ap=[[0, P], list(vb_d.ap[1])],
    )
    nc.gpsimd.dma_start(out=vb, in_=vb_bcast)

    # --- phase A: load + groupnorm for both local batches ---
    xn_tiles = []
    for b in range(BL):
        xt = xpool.tile([P, CT, N], F32, tag="xt")
        nc.sync.dma_start(out=xt, in_=x_d[b].rearrange("(t p) n -> p t n", p=P))

        stats8 = small.tile([P, 2 * CT], F32, tag="stats8")
        for ct in range(CT):
            bs = small.tile([P, 2, 6], F32, tag="bnstats")
            for h in range(2):
                nc.vector.bn_stats(
                    out=bs[:, h], in_=xt[:, ct, h * 512 : (h + 1) * 512]
                )
            mv = small.tile([P, 2], F32, tag="bnaggr")
            nc.vector.bn_aggr(out=mv, in_=bs)
            # mean into col ct; E[x^2] = mean^2 + var into col CT+ct
            nc.vector.tensor_copy(out=stats8[:, ct : ct + 1], in_=mv[:, 0:1])
            nc.vector.scalar_tensor_tensor(
                out=stats8[:, CT + ct : CT + ct + 1],
                in0=mv[:, 0:1],
                scalar=mv[:, 0:1],
                in1=mv[:, 1:2],
                op0=mybir.AluOpType.mult,
                op1=mybir.AluOpType.add,
            )

        # group-average broadcast back to channel layout: one tiny matmul
        pg_full = psum.tile([P, 512], F32, tag="mm", name="pg")
        pg = pg_full[:, : 2 * CT]
        nc.tensor.matmul(pg, lhsT=gmat, rhs=stats8, start=True, stop=True)

        ex2 = pg[:, CT : 2 * CT]
        mu = small.tile([P, CT], F32, tag="mu")
        nc.vector.tensor_copy(out=mu, in_=pg[:, 0:CT])
        var = small.tile([P, CT], F32, tag="var")
        musq = small.tile([P, CT], F32, tag="musq")
        nc.vector.tensor_mul(out=musq, in0=mu, in1=mu)
        nc.vector.tensor_sub(out=var, in0=ex2, in1=musq)
        sd = small.tile([P, CT], F32, tag="sd")
        nc.scalar.activation(
            out=sd, in_=var, func=mybir.ActivationFunctionType.Sqrt, bias=epsc
        )
        rstd = small.tile([P, CT], F32, tag="rstd")
        nc.vector.reciprocal(out=rstd, in_=sd)
        a44 = small.tile([P, CT], F32, tag="a44")
        nc.vector.tensor_mul(out=a44, in0=rstd, in1=gnw)
        tmp44 = small.tile([P, CT], F32, tag="tmp44")
        nc.vector.tensor_mul(out=tmp44, in0=mu, in1=a44)
        d44 = small.tile([P, CT], F32, tag="d44")
        nc.vector.tensor_sub(out=d44, in0=gnb, in1=tmp44)

        xn = xnpool.tile([P, CT, N], mmdt, tag="xn")
        for ct in range(CT):
            nc.vector.tensor_scalar(
                out=xn[:, ct],
                in0=xt[:, ct],
                scalar1=a44[:, ct : ct + 1],
                scalar2=d44[:, ct : ct + 1],
                op0=mybir.AluOpType.mult,
                op1=mybir.AluOpType.add,
            )
        xn_tiles.append(xn)

    # --- phase B: attention per batch ---
    for b in range(BL):
        xn = xn_tiles[b]

        # q, k in [c, n] layout (mm dtype), bias + qk-scale fused on copy-out
        q = qkpool.tile([P, CT, N], mmdt, tag="q")
        k = qkpool.tile([P, CT, N], mmdt, tag="k")
        for ct in range(CT):
            for h in range(NCH):
                sl = slice(h * 512, (h + 1) * 512)
                pq = psum.tile([P, 512], F32, tag="mm")
                for kc in range(CT):
                    nc.tensor.matmul(
                        pq,
                        lhsT=_mm(wqkv[:, kc, ct * P : (ct + 1) * P]),
                        rhs=_mm(xn[:, kc, sl]),
                        start=(kc == 0),
                        stop=(kc == CT - 1),
                    )
                nc.vector.tensor_scalar(
                    out=q[:, ct, sl],
                    in0=pq,
                    scalar1=qb[:, ct : ct + 1],
                    scalar2=QK_SCALE,
                    op0=mybir.AluOpType.add,
                    op1=mybir.AluOpType.mult,
                )
                pk = psum.tile([P, 512], F32, tag="mm")
                for kc in range(CT):
                    nc.tensor.matmul(
                        pk,
                        lhsT=_mm(wqkv[:, kc, C + ct * P : C + (ct + 1) * P]),
                        rhs=_mm(xn[:, kc, sl]),
                        start=(kc == 0),
                        stop=(kc == CT - 1),
                    )
                nc.vector.tensor_scalar(
                    out=k[:, ct, sl],
                    in0=pk,
                    scalar1=kb[:, ct : ct + 1],
                    scalar2=None,
                    op0=mybir.AluOpType.add,
                )

        # vT in [m, c] layout (always bf16: feeds the bf16 attn@v matmul)
        vT = vpool.tile([P, NT, C], BF16, tag="vT")
        for mt in range(NT):
            pv = psum.tile([P, 512], F32, tag="mm")
            for kc in range(CT):
                nc.tensor.matmul(
                    pv,
                    lhsT=_mm(xn[:, kc, mt * P : (mt + 1) * P]),
                    rhs=_mm(wqkv[:, kc, 2 * C : 3 * C]),
                    start=(kc == 0),
                    stop=(kc == CT - 1),
                )
            nc.vector.tensor_add(out=vT[:, mt], in0=pv, in1=vb)

        # S = q^T k per token tile; exp with accumulated row sum; prescale by
        # 1/sum; store P bf16; transpose 128x128 tiles via DMA xbar.
        pmat = ppool.tile([P, NT, N], BF16, tag="P")
        pmatT = ppool.tile([P, NT, N], BF16, tag="PT")
        for nt in range(NT):
            acc = small.tile([P, NCH], F32, tag="acc")
            pss = []
            for h in range(NCH):
                ps = psum.tile([P, 512], F32, tag="mm")
                for kc in range(CT):
                    nc.tensor.matmul(
                        ps,
                        lhsT=_mm(q[:, kc, nt * P : (nt + 1) * P]),
                        rhs=_mm(k[:, kc, h * 512 : (h + 1) * 512]),
                        start=(kc == 0),
                        stop=(kc == CT - 1),
                    )
                pss.append(ps)
            for h in range(NCH):
                nc.scalar.activation(
                    out=pmat[:, nt, h * 512 : (h + 1) * 512],
                    in_=pss[h],
                    func=mybir.ActivationFunctionType.Exp,
                    bias=zeroc,
                    scale=1.0,
                    accum_out=acc[:, h : h + 1],
                )
            lsum = small.tile([P, 1], F32, tag="lsum")
            nc.vector.tensor_add(out=lsum, in0=acc[:, 0:1], in1=acc[:, 1:2])
            rsum = small.tile([P, 1], F32, tag="rsum")
            nc.vector.reciprocal(out=rsum, in_=lsum)
            nc.vector.tensor_scalar_mul(pmat[:, nt], pmat[:, nt], rsum)
            for mt in range(NT):
                nc.sync.dma_start(
                    out=pmatT[:, mt, nt * P : (nt + 1) * P],
                    in_=pmat[:, nt, mt * P : (mt + 1) * P],
                    transpose=True,
                )

        # o = attn @ v in [c, n] layout (bf16 operands)
        o = opool.tile([P, CT, N], mmdt, tag="o")
        for ct in range(CT):
            for h in range(NCH):
                sl = slice(h * 512, (h + 1) * 512)
                po = psum.tile([P, 512], F32, tag="mm")
                for mt in range(NT):
                    nc.tensor.matmul(
                        po,
                        lhsT=vT[:, mt, ct * P : (ct + 1) * P],
                        rhs=pmatT[:, mt, sl],
                        start=(mt == 0),
                        stop=(mt == NT - 1),
                    )
                nc.vector.tensor_copy(out=o[:, ct, sl], in_=po)

        # out = (Wproj + I) @ o + pb  (residual folded into the weight)
        yt = ypool.tile([P, CT, N], F32, tag="yt")
        for ct in range(CT):
            for h in range(NCH):
                sl = slice(h * 512, (h + 1) * 512)
                pp = psum.tile([P, 512], F32, tag="mm")
                for kc in range(CT):
                    nc.tensor.matmul(
                        pp,
                        lhsT=_mm(wproj[:, kc, ct * P : (ct + 1) * P]),
                        rhs=_mm(o[:, kc, sl]),
                        start=(kc == 0),
                        stop=(kc == CT - 1),
                    )
                nc.scalar.activation(
                    out=yt[:, ct, sl],
                    in_=pp,
                    func=mybir.ActivationFunctionType.Identity,
                    bias=pb[:, ct : ct + 1],
                )
        nc.sync.dma_start(
            out=y_d[b].rearrange("(t p) n -> p t n", p=P), in_=yt
        )

    ctx.close()


def _legalize_waits(nc, max_waits=1):
    """This toolchain's walrus accepts at most one sync-wait command per
    instruction; Tile emits several. Spill extra waits onto standalone
    same-engine EventSemaphore carriers placed just before the instruction
    (and before its preceding LDWEIGHTS run, which walrus fuses into the
    matmul)."""
    n_carriers = 0
    for fn in nc.m.functions:
        for bb in fn.blocks:
            out = []
            changed = False
            for inst in bb.instructions:
                si = inst.sync_info
                waits = list(si.on_wait) if si is not None else []
                if len(waits) > max_waits:
                    changed = True
                    inst.sync_info = mybir.SyncInfo(
                        on_wait=waits[:max_waits], on_update=list(si.on_update)
                    )
                    carriers = []
                    for w in waits[max_waits:]:
                        n_carriers += 1
                        c = mybir.InstEventSemaphore(
                            name=f"WS-{n_carriers}", ins=[], outs=[]
                        )
                        c.engine = inst.engine
                        c.sync_info = mybir.SyncInfo(on_wait=[w], on_update=[])
                        carriers.append(c)
                    j = len(out)
                    while (
                        j > 0
                        and type(out[j - 1]).__name__ == "InstLdweights"
                        and out[j - 1].engine == inst.engine
                    ):
                        j -= 1
                    out[j:j] = carriers
                out.append(inst)
            if changed:
                bb.instructions = out
    return nc


def build(legalize=True):
    _apply_tile_patch()
    nc = bass.Bass(
        "TRN2", target_bir_lowering=False, debug=False, num_devices=N_CORES
    )
    with tile.TileContext(nc) as tc:
        _emit(tc)
    if legalize:
        _legalize_waits(nc)
    return nc


# ---------------------------------------------------------------------------
# Host-side entry point
# ---------------------------------------------------------------------------
def _host_inputs(x, gn_weight, gn_bias, qkv_weight, qkv_bias, proj_weight,
                 proj_bias):
    """Build the per-core input maps (shard x over batch; replicate weights)."""
    x = np.asarray(x, dtype=np.float32).reshape(B, C, N)
    qkv_weight = np.asarray(qkv_weight, dtype=np.float32)
    proj_weight = np.asarray(proj_weight, dtype=np.float32)

    def p44(v):
        return np.ascontiguousarray(
            np.asarray(v, dtype=np.float32).reshape(CT, P).T
        )

    wqkvT = np.ascontiguousarray(qkv_weight.T)                     # [C, 3C]
    wprojT = np.ascontiguousarray(
        (proj_weight + np.eye(C, dtype=np.float32)).T
    )                                                              # [C, C]
    gmat = np.zeros((P, P), dtype=np.float32)
    for g in range(P // GS):
        gmat[g * GS : (g + 1) * GS, g * GS : (g + 1) * GS] = 1.0 / (GS * N)
    # bn path aggregates per-channel stats (each over N elements), so the
    # group matmul just averages GS channels -> weight 1/GS.
    gmat *= float(N)  # 1/(GS*N) * N = 1/GS

    qkv_bias = np.asarray(qkv_bias, dtype=np.float32)
    shared = {
        "wqkvT": wqkvT,
        "wprojT": wprojT,
        "gnw44": p44(gn_weight),
        "gnb44": p44(gn_bias),
        "qb44": p44(qkv_bias[0:C]),
        "kb44": p44(qkv_bias[C : 2 * C]),
        "pb44": p44(proj_bias),
        "vbrow": np.ascontiguousarray(
            qkv_bias[2 * C : 3 * C].reshape(1, C)
        ),
        "gmat": gmat,
    }
    in_maps = []
    for i in range(N_CORES):
        m = dict(shared)
        m["x"] = np.ascontiguousarray(x[i * BL : (i + 1) * BL])
        in_maps.append(m)
    return in_maps


_NC = None


def kernel(x, gn_weight, gn_bias, qkv_weight, qkv_bias, proj_weight,
           proj_bias, _trace=False, _results=None):
    from concourse.bass_utils import run_bass_kernel_spmd

    global _NC
    if _NC is None:
        _NC = build()
    in_maps = _host_inputs(
        x, gn_weight, gn_bias, qkv_weight, qkv_bias, proj_weight, proj_bias
    )
    res = run_bass_kernel_spmd(
        _NC, in_maps, core_ids=list(range(N_CORES)), trace=_trace
    )
    if _results is not None:
        _results.append(res)
    y = np.concatenate([r["y"] for r in res.results], axis=0)
    return y.reshape(B, C, H, W).astype(np.float32)


# revision 8
# speedup vs baseline: 22.8980x; 22.8980x over previous
"""Trainium2 Bass kernel for nn_Attention_27874337751091.

Dense single-head attention block (GroupNorm -> qkv 1x1 conv -> softmax
attention over N=H*W tokens -> proj with residual-to-attention-output).

Sharding: data-parallel over batch B=16 across 8 NeuronCores (2 batches per
core). Weights replicated; no collectives. Each core runs an identical
program on its batch shard; the host gathers by concatenation.

Per-core pipeline (per batch, activations kept as [C, N] channel-major):
  - GroupNorm stats via bn_stats/bn_aggr per channel + a tiny PE matmul with
    a block-diagonal group-averaging matrix to broadcast group stats back to
    channels, then one fused scale+shift DVE pass.
  - q,k in [c,n] layout, v directly transposed to [m,c] (by swapping matmul
    operands), so the attention output matmul needs no extra transpose of v.
  - S = (q*C^-1/4-ish scale folded into q) ^T k accumulated in PSUM, exp on
    ScalarE with accumulated row-sum, rows pre-scaled by 1/sum, P stored
    bf16 and transposed 128x128 via DMA-xbar for the attn@v matmul.
  - proj residual is folded into the proj weight on the host (W' = W + I).
"""

from contextlib import ExitStack

import numpy as np

import concourse.bass as bass
import concourse.mybir as mybir
import concourse.tile as tile
from concourse.vector_clock import ScopedClock

# ---------------------------------------------------------------------------
# Problem constants (hardcoded per the grading contract)
# ---------------------------------------------------------------------------
N_CORES = 8
B, C, H, W = 16, 512, 32, 32
N = H * W                      # 1024 tokens
BL = B // N_CORES              # 2 batches per core
G = 32                         # groupnorm groups
GS = C // G                    # 16 channels per group
EPS = 1e-5
P = 128                        # partitions
CT = C // P                    # 4 channel tiles
NT = N // P                    # 8 token tiles
NCH = 2                        # moving-dim chunks of 512 over N
QK_SCALE = float(C) ** -0.5

F32 = mybir.dt.float32
BF16 = mybir.dt.bfloat16

# matmul input dtype mode: "bf16" (cast activations/weights to bf16) or
# "f32r" (keep fp32 on SBUF, feed the PE with float32r-bitcast APs; the
# P/PT/vT attention-probability path stays bf16 for the DMA transpose).
MM_MODE = "bf16"


# ---------------------------------------------------------------------------
# Toolchain workaround: walrus in this environment accepts at most one
# sync-wait command on a Drain, but TileContext._drain_and_barrier attaches
# the whole global clock to a single Drain. Spread the waits over individual
# SP wait_ge instructions instead.
# ---------------------------------------------------------------------------
def _patched_drain_and_barrier(self, tick_clock, wait_clock):
    nc = self.nc
    drain_inst = nc.sync.drain()
    wait_clock.add_sem_waits(
        drain_inst.ins, ScopedClock({None: tick_clock.global_clock})
    )
    si = drain_inst.ins.sync_info
    waits = list(si.on_wait) if si is not None else []
    if len(waits) > 1:
        drain_inst.ins.sync_info = mybir.SyncInfo(
            on_wait=[waits[0]], on_update=list(si.on_update)
        )
        byname = {}
        for h in wait_clock.sems.allocated().values():
            byname[getattr(h, "name", None)] = h
        for w in waits[1:]:
            nc.sync.wait_ge(byname[w.ant_name], w.wait_value)

    nc.all_engine_barrier()
    assert self.sems is not None
    popped = nc._tile_sem_poison_stack.pop()
    assert popped is self._sem_poison
    nc.clear_and_free_semaphores(list(self.sems.allocated().values()))
    nc.all_engine_barrier()


def _apply_tile_patch():
    if not getattr(tile.TileContext, "_ant_drain_patch", False):
        tile.TileContext._drain_and_barrier = _patched_drain_and_barrier
        tile.TileContext._ant_drain_patch = True


# ---------------------------------------------------------------------------
# Kernel body
# ---------------------------------------------------------------------------
def _mm(ap):
    """View an SBUF AP in the PE input dtype for MM_MODE=f32r."""
    if MM_MODE == "f32r":
        return ap.bitcast(mybir.dt.float32r)
    return ap


def _declare_io(nc):
    io = {}
    io["x"] = nc.dram_tensor("x", [BL, C, N], F32, kind="ExternalInput").ap()
    io["wqkv"] = nc.dram_tensor("wqkvT", [C, 3 * C], F32, kind="ExternalInput").ap()
    io["wproj"] = nc.dram_tensor("wprojT", [C, C], F32, kind="ExternalInput").ap()
    io["gnw"] = nc.dram_tensor("gnw44", [P, CT], F32, kind="ExternalInput").ap()
    io["gnb"] = nc.dram_tensor("gnb44", [P, CT], F32, kind="ExternalInput").ap()
    io["qb"] = nc.dram_tensor("qb44", [P, CT], F32, kind="ExternalInput").ap()
    io["kb"] = nc.dram_tensor("kb44", [P, CT], F32, kind="ExternalInput").ap()
    io["pb"] = nc.dram_tensor("pb44", [P, CT], F32, kind="ExternalInput").ap()
    io["vb"] = nc.dram_tensor("vbrow", [1, C], F32, kind="ExternalInput").ap()
    io["gmat"] = nc.dram_tensor("gmat", [P, P], F32, kind="ExternalInput").ap()
    io["y"] = nc.dram_tensor("y", [BL, C, N], F32, kind="ExternalOutput").ap()
    return io


def _emit(tc, io, rt=""):
    nc = tc.nc
    mmdt = BF16 if MM_MODE == "bf16" else F32

    x_d = io["x"]
    wqkv_d = io["wqkv"]
    wproj_d = io["wproj"]
    gnw_d = io["gnw"]
    gnb_d = io["gnb"]
    qb_d = io["qb"]
    kb_d = io["kb"]
    pb_d = io["pb"]
    vb_d = io["vb"]
    gmat_d = io["gmat"]
    y_d = io["y"]

    ctx = ExitStack()
    consts = ctx.enter_context(tc.tile_pool(name="consts" + rt, bufs=1))
    xpool = ctx.enter_context(tc.tile_pool(name="xpool" + rt, bufs=2))
    xnpool = ctx.enter_context(tc.tile_pool(name="xnpool" + rt, bufs=2))
    qkpool = ctx.enter_context(tc.tile_pool(name="qkpool" + rt, bufs=1))
    vpool = ctx.enter_context(tc.tile_pool(name="vpool" + rt, bufs=1))
    ppool = ctx.enter_context(tc.tile_pool(name="ppool" + rt, bufs=1))
    opool = ctx.enter_context(tc.tile_pool(name="opool" + rt, bufs=1))
    ypool = ctx.enter_context(tc.tile_pool(name="ypool" + rt, bufs=2))
    small = ctx.enter_context(tc.tile_pool(name="small" + rt, bufs=4))
    psum = ctx.enter_context(tc.tile_pool(name="psum" + rt, bufs=8, space="PSUM"))

    # --- constants ---
    wqkv = consts.tile([P, CT, 3 * C], mmdt, tag="wqkv")
    wqkv_src = wqkv_d.rearrange("(t p) o -> p t o", p=P)
    if mmdt == F32:
        nc.sync.dma_start(out=wqkv, in_=wqkv_src)
    else:
        nc.gpsimd.dma_start(out=wqkv, in_=wqkv_src)  # SWDGE casts f32->bf16
    wproj = consts.tile([P, CT, C], mmdt, tag="wproj")
    wproj_src = wproj_d.rearrange("(t p) o -> p t o", p=P)
    if mmdt == F32:
        nc.sync.dma_start(out=wproj, in_=wproj_src)
    else:
        nc.gpsimd.dma_start(out=wproj, in_=wproj_src)

    gnw = consts.tile([P, CT], F32, tag="gnw")
    nc.sync.dma_start(out=gnw, in_=gnw_d)
    gnb = consts.tile([P, CT], F32, tag="gnb")
    nc.sync.dma_start(out=gnb, in_=gnb_d)
    qb = consts.tile([P, CT], F32, tag="qb")
    nc.sync.dma_start(out=qb, in_=qb_d)
    kb = consts.tile([P, CT], F32, tag="kb")
    nc.sync.dma_start(out=kb, in_=kb_d)
    pb = consts.tile([P, CT], F32, tag="pb")
    nc.sync.dma_start(out=pb, in_=pb_d)
    gmat = consts.tile([P, P], F32, tag="gmat")
    nc.sync.dma_start(out=gmat, in_=gmat_d)
    epsc = consts.tile([P, 1], F32, tag="epsc")
    nc.vector.memset(epsc, EPS)
    zeroc = consts.tile([P, 1], F32, tag="zeroc")
    nc.vector.memset(zeroc, 0.0)
    # v-bias broadcast across partitions (DMA partition-step-0 replication)
    vb = consts.tile([P, C], F32, tag="vb")
    vb_bcast = bass.AP(
        tensor=vb_d.tensor,
        offset=vb_d.offset,
        ap=[[0, P], list(vb_d.ap[1])],
    )
    nc.gpsimd.dma_start(out=vb, in_=vb_bcast)

    # --- phase A: load + groupnorm for both local batches ---
    xn_tiles = []
    for b in range(BL):
        xt = xpool.tile([P, CT, N], F32, tag="xt")
        nc.sync.dma_start(out=xt, in_=x_d[b].rearrange("(t p) n -> p t n", p=P))

        stats8 = small.tile([P, 2 * CT], F32, tag="stats8")
        for ct in range(CT):
            bs = small.tile([P, 2, 6], F32, tag="bnstats")
            for h in range(2):
                nc.vector.bn_stats(
                    out=bs[:, h], in_=xt[:, ct, h * 512 : (h + 1) * 512]
                )
            mv = small.tile([P, 2], F32, tag="bnaggr")
            nc.vector.bn_aggr(out=mv, in_=bs)
            # mean into col ct; E[x^2] = mean^2 + var into col CT+ct
            nc.vector.tensor_copy(out=stats8[:, ct : ct + 1], in_=mv[:, 0:1])
            nc.vector.scalar_tensor_tensor(
                out=stats8[:, CT + ct : CT + ct + 1],
                in0=mv[:, 0:1],
                scalar=mv[:, 0:1],
                in1=mv[:, 1:2],
                op0=mybir.AluOpType.mult,
                op1=mybir.AluOpType.add,
            )

        # group-average broadcast back to channel layout: one tiny matmul
        pg_full = psum.tile([P, 512], F32, tag="mm", name="pg")
        pg = pg_full[:, : 2 * CT]
        nc.tensor.matmul(pg, lhsT=gmat, rhs=stats8, start=True, stop=True)

        ex2 = pg[:, CT : 2 * CT]
        mu = small.tile([P, CT], F32, tag="mu")
        nc.vector.tensor_copy(out=mu, in_=pg[:, 0:CT])
        var = small.tile([P, CT], F32, tag="var")
        musq = small.tile([P, CT], F32, tag="musq")
        nc.vector.tensor_mul(out=musq, in0=mu, in1=mu)
        nc.vector.tensor_sub(out=var, in0=ex2, in1=musq)
        sd = small.tile([P, CT], F32, tag="sd")
        nc.scalar.activation(
            out=sd, in_=var, func=mybir.ActivationFunctionType.Sqrt, bias=epsc
        )
        rstd = small.tile([P, CT], F32, tag="rstd")
        nc.vector.reciprocal(out=rstd, in_=sd)
        a44 = small.tile([P, CT], F32, tag="a44")
        nc.vector.tensor_mul(out=a44, in0=rstd, in1=gnw)
        tmp44 = small.tile([P, CT], F32, tag="tmp44")
        nc.vector.tensor_mul(out=tmp44, in0=mu, in1=a44)
        d44 = small.tile([P, CT], F32, tag="d44")
        nc.vector.tensor_sub(out=d44, in0=gnb, in1=tmp44)

        xn = xnpool.tile([P, CT, N], mmdt, tag="xn")
        for ct in range(CT):
            nc.vector.tensor_scalar(
                out=xn[:, ct],
                in0=xt[:, ct],
                scalar1=a44[:, ct : ct + 1],
                scalar2=d44[:, ct : ct + 1],
                op0=mybir.AluOpType.mult,
                op1=mybir.AluOpType.add,
            )
        xn_tiles.append(xn)

    # --- phase B: attention per batch ---
    for b in range(BL):
        xn = xn_tiles[b]

        # q, k in [c, n] layout (mm dtype), bias + qk-scale fused on copy-out
        q = qkpool.tile([P, CT, N], mmdt, tag="q")
        k = qkpool.tile([P, CT, N], mmdt, tag="k")
        for ct in range(CT):
            for h in range(NCH):
                sl = slice(h * 512, (h + 1) * 512)
                pq = psum.tile([P, 512], F32, tag="mm")
                for kc in range(CT):
                    nc.tensor.matmul(
                        pq,
                        lhsT=_mm(wqkv[:, kc, ct * P : (ct + 1) * P]),
                        rhs=_mm(xn[:, kc, sl]),
                        start=(kc == 0),
                        stop=(kc == CT - 1),
                    )
                nc.vector.tensor_scalar(
                    out=q[:, ct, sl],
                    in0=pq,
                    scalar1=qb[:, ct : ct + 1],
                    scalar2=QK_SCALE,
                    op0=mybir.AluOpType.add,
                    op1=mybir.AluOpType.mult,
                )
                pk = psum.tile([P, 512], F32, tag="mm")
                for kc in range(CT):
                    nc.tensor.matmul(
                        pk,
                        lhsT=_mm(wqkv[:, kc, C + ct * P : C + (ct + 1) * P]),
                        rhs=_mm(xn[:, kc, sl]),
                        start=(kc == 0),
                        stop=(kc == CT - 1),
                    )
                nc.vector.tensor_scalar(
                    out=k[:, ct, sl],
                    in0=pk,
                    scalar1=kb[:, ct : ct + 1],
                    scalar2=None,
                    op0=mybir.AluOpType.add,
                )

        # vT in [m, c] layout (always bf16: feeds the bf16 attn@v matmul)
        vT = vpool.tile([P, NT, C], BF16, tag="vT")
        for mt in range(NT):
            pv = psum.tile([P, 512], F32, tag="mm")
            for kc in range(CT):
                nc.tensor.matmul(
                    pv,
                    lhsT=_mm(xn[:, kc, mt * P : (mt + 1) * P]),
                    rhs=_mm(wqkv[:, kc, 2 * C : 3 * C]),
                    start=(kc == 0),
                    stop=(kc == CT - 1),
                )
            nc.vector.tensor_add(out=vT[:, mt], in0=pv, in1=vb)

        # S = q^T k per token tile; exp with accumulated row sum; prescale by
        # 1/sum; store P bf16; transpose 128x128 tiles via DMA xbar.
        pmat = ppool.tile([P, NT, N], BF16, tag="P")
        pmatT = ppool.tile([P, NT, N], BF16, tag="PT")
        for nt in range(NT):
            acc = small.tile([P, NCH], F32, tag="acc")
            pss = []
            for h in range(NCH):
                ps = psum.tile([P, 512], F32, tag="mm")
                for kc in range(CT):
                    nc.tensor.matmul(
                        ps,
                        lhsT=_mm(q[:, kc, nt * P : (nt + 1) * P]),
                        rhs=_mm(k[:, kc, h * 512 : (h + 1) * 512]),
                        start=(kc == 0),
                        stop=(kc == CT - 1),
                    )
                pss.append(ps)
            for h in range(NCH):
                nc.scalar.activation(
                    out=pmat[:, nt, h * 512 : (h + 1) * 512],
                    in_=pss[h],
                    func=mybir.ActivationFunctionType.Exp,
                    bias=zeroc,
                    scale=1.0,
                    accum_out=acc[:, h : h + 1],
                )
            lsum = small.tile([P, 1], F32, tag="lsum")
            nc.vector.tensor_add(out=lsum, in0=acc[:, 0:1], in1=acc[:, 1:2])
            rsum = small.tile([P, 1], F32, tag="rsum")
            nc.vector.reciprocal(out=rsum, in_=lsum)
            nc.vector.tensor_scalar_mul(pmat[:, nt], pmat[:, nt], rsum)
            for mt in range(NT):
                nc.sync.dma_start(
                    out=pmatT[:, mt, nt * P : (nt + 1) * P],
                    in_=pmat[:, nt, mt * P : (mt + 1) * P],
                    transpose=True,
                )

        # o = attn @ v in [c, n] layout (bf16 operands)
        o = opool.tile([P, CT, N], mmdt, tag="o")
        for ct in range(CT):
            for h in range(NCH):
                sl = slice(h * 512, (h + 1) * 512)
                po = psum.tile([P, 512], F32, tag="mm")
                for mt in range(NT):
                    nc.tensor.matmul(
                        po,
                        lhsT=vT[:, mt, ct * P : (ct + 1) * P],
                        rhs=pmatT[:, mt, sl],
                        start=(mt == 0),
                        stop=(mt == NT - 1),
                    )
                nc.vector.tensor_copy(out=o[:, ct, sl], in_=po)

        # out = (Wproj + I) @ o + pb  (residual folded into the weight)
        yt = ypool.tile([P, CT, N], F32, tag="yt")
        for ct in range(CT):
            for h in range(NCH):
                sl = slice(h * 512, (h + 1) * 512)
                pp = psum.tile([P, 512], F32, tag="mm")
                for kc in range(CT):
                    nc.tensor.matmul(
                        pp,
                        lhsT=_mm(wproj[:, kc, ct * P : (ct + 1) * P]),
                        rhs=_mm(o[:, kc, sl]),
                        start=(kc == 0),
                        stop=(kc == CT - 1),
                    )
                nc.scalar.activation(
                    out=yt[:, ct, sl],
                    in_=pp,
                    func=mybir.ActivationFunctionType.Identity,
                    bias=pb[:, ct : ct + 1],
                )
        nc.sync.dma_start(
            out=y_d[b].rearrange("(t p) n -> p t n", p=P), in_=yt
        )

    ctx.close()


def _legalize_waits(nc, max_waits=1):
    """This toolchain's walrus accepts at most one sync-wait command per
    instruction; Tile emits several. Spill extra waits onto standalone
    same-engine EventSemaphore carriers placed just before the instruction
    (and before its preceding LDWEIGHTS run, which walrus fuses into the
    matmul)."""
    n_carriers = 0
    for fn in nc.m.functions:
        for bb in fn.blocks:
            out = []
            changed = False
            for inst in bb.instructions:
                si = inst.sync_info
                waits = list(si.on_wait) if si is not None else []
                if len(waits) > max_waits:
                    changed = True
                    inst.sync_info = mybir.SyncInfo(
                        on_wait=waits[:max_waits], on_update=list(si.on_update)
                    )
                    carriers = []
                    for w in waits[max_waits:]:
                        n_carriers += 1
                        c = mybir.InstEventSemaphore(
                            name=f"WS-{n_carriers}", ins=[], outs=[]
                        )
                        c.engine = inst.engine
                        c.sync_info = mybir.SyncInfo(on_wait=[w], on_update=[])
                        carriers.append(c)
                    j = len(out)
                    while (
                        j > 0
                        and type(out[j - 1]).__name__ == "InstLdweights"
                        and out[j - 1].engine == inst.engine
                    ):
                        j -= 1
                    out[j:j] = carriers
                out.append(inst)
            if changed:
                bb.instructions = out
    return nc


def build(legalize=True, reps=1):
    _apply_tile_patch()
    nc = bass.Bass(
        "TRN2", target_bir_lowering=False, debug=False, num_devices=N_CORES
    )
    with tile.TileContext(nc) as tc:
        io = _declare_io(nc)
        for r in range(reps):
            _emit(tc, io, rt=f"_{r}" if r else "")
    if legalize:
        _legalize_waits(nc)
    return nc


# ---------------------------------------------------------------------------
# Host-side entry point
# ---------------------------------------------------------------------------
def _host_inputs(x, gn_weight, gn_bias, qkv_weight, qkv_bias, proj_weight,
                 proj_bias):
    """Build the per-core input maps (shard x over batch; replicate weights)."""
    x = np.asarray(x, dtype=np.float32).reshape(B, C, N)
    qkv_weight = np.asarray(qkv_weight, dtype=np.float32)
    proj_weight = np.asarray(proj_weight, dtype=np.float32)

    def p44(v):
        return np.ascontiguousarray(
            np.asarray(v, dtype=np.float32).reshape(CT, P).T
        )

    wqkvT = np.ascontiguousarray(qkv_weight.T)                     # [C, 3C]
    wprojT = np.ascontiguousarray(
        (proj_weight + np.eye(C, dtype=np.float32)).T
    )                                                              # [C, C]
    gmat = np.zeros((P, P), dtype=np.float32)
    for g in range(P // GS):
        gmat[g * GS : (g + 1) * GS, g * GS : (g + 1) * GS] = 1.0 / (GS * N)
    # bn path aggregates per-channel stats (each over N elements), so the
    # group matmul just averages GS channels -> weight 1/GS.
    gmat *= float(N)  # 1/(GS*N) * N = 1/GS

    qkv_bias = np.asarray(qkv_bias, dtype=np.float32)
    shared = {
        "wqkvT": wqkvT,
        "wprojT": wprojT,
        "gnw44": p44(gn_weight),
        "gnb44": p44(gn_bias),
        "qb44": p44(qkv_bias[0:C]),
        "kb44": p44(qkv_bias[C : 2 * C]),
        "pb44": p44(proj_bias),
        "vbrow": np.ascontiguousarray(
            qkv_bias[2 * C : 3 * C].reshape(1, C)
        ),
        "gmat": gmat,
    }
    in_maps = []
    for i in range(N_CORES):
        m = dict(shared)
        m["x"] = np.ascontiguousarray(x[i * BL : (i + 1) * BL])
        in_maps.append(m)
    return in_maps


_NC = None


def kernel(x, gn_weight, gn_bias, qkv_weight, qkv_bias, proj_weight,
           proj_bias, _trace=False, _results=None):
    from concourse.bass_utils import run_bass_kernel_spmd

    global _NC
    if _NC is None:
        _NC = build()
    in_maps = _host_inputs(
        x, gn_weight, gn_bias, qkv_weight, qkv_bias, proj_weight, proj_bias
    )
    res = run_bass_kernel_spmd(
        _NC, in_maps, core_ids=list(range(N_CORES)), trace=_trace
    )
    if _results is not None:
        _results.append(res)
    y = np.concatenate([r["y"] for r in res.results], axis=0)
    return y.reshape(B, C, H, W).astype(np.float32)


# revision 12
# speedup vs baseline: 41.2569x; 1.8018x over previous
"""Trainium2 Bass kernel for nn_Attention_27874337751091.

Dense single-head attention block (GroupNorm -> qkv 1x1 conv -> softmax
attention over N=H*W tokens -> proj with residual-to-attention-output).

Sharding: data-parallel over batch B=16 across 8 NeuronCores (2 batches per
core). Weights replicated; no collectives. Each core runs an identical
program on its batch shard; the host gathers by concatenation.

Per-core pipeline (per batch, activations kept as [C, N] channel-major):
  - GroupNorm stats via bn_stats/bn_aggr per channel + a tiny PE matmul with
    a block-diagonal group-averaging matrix to broadcast group stats back to
    channels, then one fused scale+shift DVE pass.
  - q,k in [c,n] layout, v directly transposed to [m,c] (by swapping matmul
    operands), so the attention output matmul needs no extra transpose of v.
  - S = (q*C^-1/4-ish scale folded into q) ^T k accumulated in PSUM, exp on
    ScalarE with accumulated row-sum, rows pre-scaled by 1/sum, P stored
    bf16 and transposed 128x128 via DMA-xbar for the attn@v matmul.
  - proj residual is folded into the proj weight on the host (W' = W + I).
"""

from contextlib import ExitStack

import numpy as np

import concourse.bass as bass
import concourse.mybir as mybir
import concourse.tile as tile
from concourse.vector_clock import ScopedClock

# ---------------------------------------------------------------------------
# Problem constants (hardcoded per the grading contract)
# ---------------------------------------------------------------------------
N_CORES = 8
B, C, H, W = 16, 512, 32, 32
N = H * W                      # 1024 tokens
BL = B // N_CORES              # 2 batches per core
G = 32                         # groupnorm groups
GS = C // G                    # 16 channels per group
EPS = 1e-5
P = 128                        # partitions
CT = C // P                    # 4 channel tiles
NT = N // P                    # 8 token tiles
NCH = 2                        # moving-dim chunks of 512 over N
QK_SCALE = float(C) ** -0.5

F32 = mybir.dt.float32
BF16 = mybir.dt.bfloat16

# matmul input dtype mode: "bf16" (cast activations/weights to bf16) or
# "f32r" (keep fp32 on SBUF, feed the PE with float32r-bitcast APs; the
# P/PT/vT attention-probability path stays bf16 for the DMA transpose).
MM_MODE = "bf16"

# "strip": one dma_start_transpose per token-tile strip (8 insts/batch);
# "tile": 64 per-128x128-tile dma transposes per batch.
TRANSPOSE_MODE = "strip"


# ---------------------------------------------------------------------------
# Toolchain workaround: walrus in this environment accepts at most one
# sync-wait command on a Drain, but TileContext._drain_and_barrier attaches
# the whole global clock to a single Drain. Spread the waits over individual
# SP wait_ge instructions instead.
# ---------------------------------------------------------------------------
def _patched_drain_and_barrier(self, tick_clock, wait_clock):
    nc = self.nc
    drain_inst = nc.sync.drain()
    wait_clock.add_sem_waits(
        drain_inst.ins, ScopedClock({None: tick_clock.global_clock})
    )
    si = drain_inst.ins.sync_info
    waits = list(si.on_wait) if si is not None else []
    if len(waits) > 1:
        drain_inst.ins.sync_info = mybir.SyncInfo(
            on_wait=[waits[0]], on_update=list(si.on_update)
        )
        byname = {}
        for h in wait_clock.sems.allocated().values():
            byname[getattr(h, "name", None)] = h
        for w in waits[1:]:
            nc.sync.wait_ge(byname[w.ant_name], w.wait_value)

    nc.all_engine_barrier()
    assert self.sems is not None
    popped = nc._tile_sem_poison_stack.pop()
    assert popped is self._sem_poison
    nc.clear_and_free_semaphores(list(self.sems.allocated().values()))
    nc.all_engine_barrier()


def _apply_tile_patch():
    if not getattr(tile.TileContext, "_ant_drain_patch", False):
        tile.TileContext._drain_and_barrier = _patched_drain_and_barrier
        tile.TileContext._ant_drain_patch = True


# ---------------------------------------------------------------------------
# Kernel body
# ---------------------------------------------------------------------------
def _mm(ap):
    """View an SBUF AP in the PE input dtype for MM_MODE=f32r."""
    if MM_MODE == "f32r":
        return ap.bitcast(mybir.dt.float32r)
    return ap


def _declare_io(nc):
    io = {}
    io["x"] = nc.dram_tensor("x", [BL, C, N], F32, kind="ExternalInput").ap()
    io["wqkv"] = nc.dram_tensor("wqkvT", [C, 3 * C], F32, kind="ExternalInput").ap()
    io["wproj"] = nc.dram_tensor("wprojT", [C, C], F32, kind="ExternalInput").ap()
    io["gnw"] = nc.dram_tensor("gnw44", [P, CT], F32, kind="ExternalInput").ap()
    io["gnb"] = nc.dram_tensor("gnb44", [P, CT], F32, kind="ExternalInput").ap()
    io["qb"] = nc.dram_tensor("qb44", [P, CT], F32, kind="ExternalInput").ap()
    io["kb"] = nc.dram_tensor("kb44", [P, CT], F32, kind="ExternalInput").ap()
    io["pb"] = nc.dram_tensor("pb44", [P, CT], F32, kind="ExternalInput").ap()
    io["vb"] = nc.dram_tensor("vbrow", [1, C], F32, kind="ExternalInput").ap()
    io["gmat"] = nc.dram_tensor("gmat", [P, P], F32, kind="ExternalInput").ap()
    io["y"] = nc.dram_tensor("y", [BL, C, N], F32, kind="ExternalOutput").ap()
    return io


def _emit(tc, io, rt=""):
    nc = tc.nc
    mmdt = BF16 if MM_MODE == "bf16" else F32

    x_d = io["x"]
    wqkv_d = io["wqkv"]
    wproj_d = io["wproj"]
    gnw_d = io["gnw"]
    gnb_d = io["gnb"]
    qb_d = io["qb"]
    kb_d = io["kb"]
    pb_d = io["pb"]
    vb_d = io["vb"]
    gmat_d = io["gmat"]
    y_d = io["y"]

    ctx = ExitStack()
    consts = ctx.enter_context(tc.tile_pool(name="consts" + rt, bufs=1))
    xpool = ctx.enter_context(tc.tile_pool(name="xpool" + rt, bufs=2))
    xnpool = ctx.enter_context(tc.tile_pool(name="xnpool" + rt, bufs=2))
    qkpool = ctx.enter_context(tc.tile_pool(name="qkpool" + rt, bufs=1))
    vpool = ctx.enter_context(tc.tile_pool(name="vpool" + rt, bufs=1))
    ppool = ctx.enter_context(tc.tile_pool(name="ppool" + rt, bufs=1))
    opool = ctx.enter_context(tc.tile_pool(name="opool" + rt, bufs=1))
    ypool = ctx.enter_context(tc.tile_pool(name="ypool" + rt, bufs=2))
    small = ctx.enter_context(tc.tile_pool(name="small" + rt, bufs=4))
    psum = ctx.enter_context(tc.tile_pool(name="psum" + rt, bufs=4, space="PSUM"))

    # --- constants ---
    wqkv = consts.tile([P, CT, 3 * C], mmdt, tag="wqkv")
    wqkv_src = wqkv_d.rearrange("(t p) o -> p t o", p=P)
    if mmdt == F32:
        nc.sync.dma_start(out=wqkv, in_=wqkv_src)
    else:
        nc.gpsimd.dma_start(out=wqkv, in_=wqkv_src)  # SWDGE casts f32->bf16
    wproj = consts.tile([P, CT, C], mmdt, tag="wproj")
    wproj_src = wproj_d.rearrange("(t p) o -> p t o", p=P)
    if mmdt == F32:
        nc.sync.dma_start(out=wproj, in_=wproj_src)
    else:
        nc.gpsimd.dma_start(out=wproj, in_=wproj_src)

    gnw = consts.tile([P, CT], F32, tag="gnw")
    nc.sync.dma_start(out=gnw, in_=gnw_d)
    gnb = consts.tile([P, CT], F32, tag="gnb")
    nc.sync.dma_start(out=gnb, in_=gnb_d)
    qb = consts.tile([P, CT], F32, tag="qb")
    nc.sync.dma_start(out=qb, in_=qb_d)
    kb = consts.tile([P, CT], F32, tag="kb")
    nc.sync.dma_start(out=kb, in_=kb_d)
    pb = consts.tile([P, CT], F32, tag="pb")
    nc.sync.dma_start(out=pb, in_=pb_d)
    gmat = consts.tile([P, P], F32, tag="gmat")
    nc.sync.dma_start(out=gmat, in_=gmat_d)
    epsc = consts.tile([P, 1], F32, tag="epsc")
    nc.vector.memset(epsc, EPS)
    zeroc = consts.tile([P, 1], F32, tag="zeroc")
    nc.vector.memset(zeroc, 0.0)
    # v-bias broadcast across partitions (DMA partition-step-0 replication)
    vb = consts.tile([P, C], F32, tag="vb")
    vb_bcast = bass.AP(
        tensor=vb_d.tensor,
        offset=vb_d.offset,
        ap=[[0, P], list(vb_d.ap[1])],
    )
    nc.gpsimd.dma_start(out=vb, in_=vb_bcast)

    # --- phase A: load + groupnorm for both local batches ---
    xn_tiles = []
    for b in range(BL):
        xt = xpool.tile([P, CT, N], F32, tag="xt")
        nc.sync.dma_start(out=xt, in_=x_d[b].rearrange("(t p) n -> p t n", p=P))

        # per-channel sums of x and x^2 via ScalarE accumulate outputs
        stats8 = small.tile([P, 2 * CT], F32, tag="stats8")
        scr = xnpool.tile([P, N], F32, tag="scr")
        for ct in range(CT):
            nc.scalar.activation(
                out=scr, in_=xt[:, ct],
                func=mybir.ActivationFunctionType.Identity,
                bias=zeroc, accum_out=stats8[:, ct : ct + 1],
            )
            nc.scalar.activation(
                out=scr, in_=xt[:, ct],
                func=mybir.ActivationFunctionType.Square,
                bias=zeroc, accum_out=stats8[:, CT + ct : CT + ct + 1],
            )

        # group-average broadcast back to channel layout: one tiny matmul
        # (gmat = blockdiag(1/(GS*N)) so columns become mu_g / E_g[x^2])
        pg_full = psum.tile([P, 2, 512], F32, tag="mm", name="pg")
        pg = pg_full[:, 0, : 2 * CT]
        nc.tensor.matmul(pg, lhsT=gmat, rhs=stats8, start=True, stop=True)

        ex2 = pg[:, CT : 2 * CT]
        mu = small.tile([P, CT], F32, tag="mu")
        nc.vector.tensor_copy(out=mu, in_=pg[:, 0:CT])
        var = small.tile([P, CT], F32, tag="var")
        musq = small.tile([P, CT], F32, tag="musq")
        nc.vector.tensor_mul(out=musq, in0=mu, in1=mu)
        nc.vector.tensor_sub(out=var, in0=ex2, in1=musq)
        sd = small.tile([P, CT], F32, tag="sd")
        nc.scalar.activation(
            out=sd, in_=var, func=mybir.ActivationFunctionType.Sqrt, bias=epsc
        )
        rstd = small.tile([P, CT], F32, tag="rstd")
        nc.vector.reciprocal(out=rstd, in_=sd)
        a44 = small.tile([P, CT], F32, tag="a44")
        nc.vector.tensor_mul(out=a44, in0=rstd, in1=gnw)
        tmp44 = small.tile([P, CT], F32, tag="tmp44")
        nc.vector.tensor_mul(out=tmp44, in0=mu, in1=a44)
        d44 = small.tile([P, CT], F32, tag="d44")
        nc.vector.tensor_sub(out=d44, in0=gnb, in1=tmp44)

        xn = xnpool.tile([P, CT, N], mmdt, tag="xn")
        for ct in range(CT):
            nc.vector.tensor_scalar(
                out=xn[:, ct],
                in0=xt[:, ct],
                scalar1=a44[:, ct : ct + 1],
                scalar2=d44[:, ct : ct + 1],
                op0=mybir.AluOpType.mult,
                op1=mybir.AluOpType.add,
            )
        xn_tiles.append(xn)

    # --- phase B: attention per batch ---
    for b in range(BL):
        xn = xn_tiles[b]

        # q, k in [c, n] layout (mm dtype), bias + qk-scale fused on copy-out
        q = qkpool.tile([P, CT, N], mmdt, tag="q")
        k = qkpool.tile([P, CT, N], mmdt, tag="k")
        for ct in range(CT):
            pq = psum.tile([P, NCH, 512], F32, tag="mm", name="pq")
            for h in range(NCH):
                for kc in range(CT):
                    nc.tensor.matmul(
                        pq[:, h],
                        lhsT=_mm(wqkv[:, kc, ct * P : (ct + 1) * P]),
                        rhs=_mm(xn[:, kc, h * 512 : (h + 1) * 512]),
                        start=(kc == 0),
                        stop=(kc == CT - 1),
                    )
            nc.vector.tensor_scalar(
                out=q[:, ct].rearrange("p (h m) -> p h m", h=NCH),
                in0=pq,
                scalar1=qb[:, ct : ct + 1],
                scalar2=QK_SCALE,
                op0=mybir.AluOpType.add,
                op1=mybir.AluOpType.mult,
            )
            pk = psum.tile([P, NCH, 512], F32, tag="mm", name="pk")
            for h in range(NCH):
                for kc in range(CT):
                    nc.tensor.matmul(
                        pk[:, h],
                        lhsT=_mm(wqkv[:, kc, C + ct * P : C + (ct + 1) * P]),
                        rhs=_mm(xn[:, kc, h * 512 : (h + 1) * 512]),
                        start=(kc == 0),
                        stop=(kc == CT - 1),
                    )
            nc.vector.tensor_scalar(
                out=k[:, ct].rearrange("p (h m) -> p h m", h=NCH),
                in0=pk,
                scalar1=kb[:, ct : ct + 1],
                scalar2=None,
                op0=mybir.AluOpType.add,
            )

        # vT in [m, c] layout (always bf16: feeds the bf16 attn@v matmul)
        vT = vpool.tile([P, NT, C], BF16, tag="vT")
        for mp in range(NT // 2):
            pv = psum.tile([P, 2, 512], F32, tag="mm", name="pv")
            for i in range(2):
                mt = 2 * mp + i
                for kc in range(CT):
                    nc.tensor.matmul(
                        pv[:, i],
                        lhsT=_mm(xn[:, kc, mt * P : (mt + 1) * P]),
                        rhs=_mm(wqkv[:, kc, 2 * C : 3 * C]),
                        start=(kc == 0),
                        stop=(kc == CT - 1),
                    )
            nc.vector.tensor_tensor(
                out=vT[:, 2 * mp : 2 * mp + 2],
                in0=pv,
                in1=vb[:, None, :].to_broadcast([P, 2, C]),
                op=mybir.AluOpType.add,
            )

        # S = q^T k per token tile; exp with accumulated row sums
        pmat = ppool.tile([P, NT, N], BF16, tag="P")
        pmatT = ppool.tile([P, NT, N], BF16, tag="PT")
        lsum8 = small.tile([P, NT], F32, tag="lsum8")
        for nt in range(NT):
            ps = psum.tile([P, NCH, 512], F32, tag="mm", name="ps")
            for h in range(NCH):
                for kc in range(CT):
                    nc.tensor.matmul(
                        ps[:, h],
                        lhsT=_mm(q[:, kc, nt * P : (nt + 1) * P]),
                        rhs=_mm(k[:, kc, h * 512 : (h + 1) * 512]),
                        start=(kc == 0),
                        stop=(kc == CT - 1),
                    )
            nc.scalar.activation(
                out=pmat[:, nt],
                in_=ps.rearrange("p h m -> p (h m)"),
                func=mybir.ActivationFunctionType.Exp,
                bias=zeroc,
                scale=1.0,
                accum_out=lsum8[:, nt : nt + 1],
            )
        rsum8 = small.tile([P, NT], F32, tag="rsum8")
        nc.vector.reciprocal(out=rsum8, in_=lsum8)
        nc.vector.tensor_tensor(
            out=pmat,
            in0=pmat,
            in1=rsum8[:, :, None].to_broadcast([P, NT, N]),
            op=mybir.AluOpType.mult,
        )
        # transpose: PT[pm, mt, nt*128+nn] = P[nn, nt, mt*128+pm]
        if TRANSPOSE_MODE == "strip":
            for nt in range(NT):
                nc.sync.dma_start_transpose(
                    out=pmatT[:, :, nt * P : (nt + 1) * P],
                    in_=pmat[:, nt],
                )
        else:
            for nt in range(NT):
                for mt in range(NT):
                    nc.sync.dma_start(
                        out=pmatT[:, mt, nt * P : (nt + 1) * P],
                        in_=pmat[:, nt, mt * P : (mt + 1) * P],
                        transpose=True,
                    )

        # o = attn @ v in [c, n] layout (bf16 operands)
        o = opool.tile([P, CT, N], mmdt, tag="o")
        for ct in range(CT):
            po = psum.tile([P, NCH, 512], F32, tag="mm", name="po")
            for h in range(NCH):
                for mt in range(NT):
                    nc.tensor.matmul(
                        po[:, h],
                        lhsT=vT[:, mt, ct * P : (ct + 1) * P],
                        rhs=pmatT[:, mt, h * 512 : (h + 1) * 512],
                        start=(mt == 0),
                        stop=(mt == NT - 1),
                    )
            nc.vector.tensor_copy(
                out=o[:, ct].rearrange("p (h m) -> p h m", h=NCH), in_=po
            )

        # out = (Wproj + I) @ o + pb  (residual folded into the weight)
        yt = ypool.tile([P, CT, N], F32, tag="yt")
        for ct in range(CT):
            pp = psum.tile([P, NCH, 512], F32, tag="mm", name="pp")
            for h in range(NCH):
                for kc in range(CT):
                    nc.tensor.matmul(
                        pp[:, h],
                        lhsT=_mm(wproj[:, kc, ct * P : (ct + 1) * P]),
                        rhs=_mm(o[:, kc, h * 512 : (h + 1) * 512]),
                        start=(kc == 0),
                        stop=(kc == CT - 1),
                    )
            nc.scalar.activation(
                out=yt[:, ct],
                in_=pp.rearrange("p h m -> p (h m)"),
                func=mybir.ActivationFunctionType.Identity,
                bias=pb[:, ct : ct + 1],
            )
        nc.sync.dma_start(
            out=y_d[b].rearrange("(t p) n -> p t n", p=P), in_=yt
        )

    ctx.close()


def _legalize_waits(nc, max_waits=1):
    """Platform tuning + legalization:

    1. Drop InstLdweights: Tile splits each self-loading matmul into a
       Ldweights prefetch + Matmult; the Matmult still carries both
       operands, so the Ldweights is redundant and on this platform costs
       a full instruction dispatch. Its waits move to the next PE inst.
    2. Walrus here accepts at most one sync-wait command per instruction;
       spill extras onto standalone same-engine EventSemaphore carriers.
    """
    n_carriers = 0
    for fn in nc.m.functions:
        for bb in fn.blocks:
            out = []
            changed = False
            pend_pe_waits = []
            for inst in bb.instructions:
                si = inst.sync_info
                waits = list(si.on_wait) if si is not None else []
                if type(inst).__name__ == "InstLdweights":
                    changed = True
                    pend_pe_waits.extend(waits)
                    continue
                if type(inst).__name__ == "InstMatmult":
                    # restore self-loading (the split Ldweights is dropped)
                    inst.ldweights = None
                if pend_pe_waits and inst.engine == mybir.EngineType.PE:
                    seen = {(w.id, w.wait_mode) for w in waits}
                    for w in pend_pe_waits:
                        if (w.id, w.wait_mode) in seen:
                            for i, ow in enumerate(waits):
                                if (ow.id, ow.wait_mode) == (w.id, w.wait_mode):
                                    if w.wait_value > ow.wait_value:
                                        waits[i] = w
                                    break
                        else:
                            waits.append(w)
                            seen.add((w.id, w.wait_mode))
                    pend_pe_waits = []
                    changed = True
                    inst.sync_info = mybir.SyncInfo(
                        on_wait=waits,
                        on_update=list(si.on_update) if si is not None else [],
                    )
                    si = inst.sync_info
                if len(waits) > max_waits:
                    changed = True
                    inst.sync_info = mybir.SyncInfo(
                        on_wait=waits[:max_waits], on_update=list(si.on_update)
                    )
                    carriers = []
                    for w in waits[max_waits:]:
                        n_carriers += 1
                        c = mybir.InstEventSemaphore(
                            name=f"WS-{n_carriers}", ins=[], outs=[]
                        )
                        c.engine = inst.engine
                        c.sync_info = mybir.SyncInfo(on_wait=[w], on_update=[])
                        carriers.append(c)
                    out.extend(carriers)
                out.append(inst)
            if changed:
                bb.instructions = out
    return nc


def build(legalize=True, reps=1):
    _apply_tile_patch()
    nc = bass.Bass(
        "TRN2", target_bir_lowering=False, debug=False, num_devices=N_CORES
    )
    with tile.TileContext(nc) as tc:
        io = _declare_io(nc)
        for r in range(reps):
            _emit(tc, io, rt=f"_{r}" if r else "")
    if legalize:
        _legalize_waits(nc)
    return nc


# ---------------------------------------------------------------------------
# Host-side entry point
# ---------------------------------------------------------------------------
def _host_inputs(x, gn_weight, gn_bias, qkv_weight, qkv_bias, proj_weight,
                 proj_bias):
    """Build the per-core input maps (shard x over batch; replicate weights)."""
    x = np.asarray(x, dtype=np.float32).reshape(B, C, N)
    qkv_weight = np.asarray(qkv_weight, dtype=np.float32)
    proj_weight = np.asarray(proj_weight, dtype=np.float32)

    def p44(v):
        return np.ascontiguousarray(
            np.asarray(v, dtype=np.float32).reshape(CT, P).T
        )

    wqkvT = np.ascontiguousarray(qkv_weight.T)                     # [C, 3C]
    wprojT = np.ascontiguousarray(
        (proj_weight + np.eye(C, dtype=np.float32)).T
    )                                                              # [C, C]
    gmat = np.zeros((P, P), dtype=np.float32)
    for g in range(P // GS):
        gmat[g * GS : (g + 1) * GS, g * GS : (g + 1) * GS] = 1.0 / (GS * N)

    qkv_bias = np.asarray(qkv_bias, dtype=np.float32)
    shared = {
        "wqkvT": wqkvT,
        "wprojT": wprojT,
        "gnw44": p44(gn_weight),
        "gnb44": p44(gn_bias),
        "qb44": p44(qkv_bias[0:C]),
        "kb44": p44(qkv_bias[C : 2 * C]),
        "pb44": p44(proj_bias),
        "vbrow": np.ascontiguousarray(
            qkv_bias[2 * C : 3 * C].reshape(1, C)
        ),
        "gmat": gmat,
    }
    in_maps = []
    for i in range(N_CORES):
        m = dict(shared)
        m["x"] = np.ascontiguousarray(x[i * BL : (i + 1) * BL])
        in_maps.append(m)
    return in_maps


_NC = None


def kernel(x, gn_weight, gn_bias, qkv_weight, qkv_bias, proj_weight,
           proj_bias, _trace=False, _results=None):
    from concourse.bass_utils import run_bass_kernel_spmd

    global _NC
    if _NC is None:
        _NC = build()
    in_maps = _host_inputs(
        x, gn_weight, gn_bias, qkv_weight, qkv_bias, proj_weight, proj_bias
    )
    res = run_bass_kernel_spmd(
        _NC, in_maps, core_ids=list(range(N_CORES)), trace=_trace
    )
    if _results is not None:
        _results.append(res)
    y = np.concatenate([r["y"] for r in res.results], axis=0)
    return y.reshape(B, C, H, W).astype(np.float32)
